# revision 10
# baseline (speedup 1.0000x reference)
"""Trainium2 Bass kernel for nn_DeepDCNN (dense CNN + k-max pooling), 8-core data parallel.

- Batch 64 sharded as 8 per core; weights/table replicated.
- Embedding gather: indirect DMA rows->partitions + PE transposes.
- Convs: conv+fold fused into fp32 tap-matmuls w/ shifted APs (stage1 pairs taps);
  bias added via a K=1 ones-matmul in the same PSUM accumulation group.
- k-max: per-row threshold by sampled bracket + fixed bisection (ACT Sign+accum
  counts, scalar state batched across chunks), snap via max8 rounds, then
  mask -> cumsum scan -> GPSIMD local_scatter of fp32 bit-planes -> tanh.
- FC on PE. Output (64, 6) fp32.
"""

import numpy as np

V, E, B, SEQ, NCLS = 50000, 64, 64, 1024, 6
NF = [10, 14, 18, 22]
KS = [7, 5, 5, 3]
GROUPS = [64, 32, 16, 8]
KPOOL = [768, 512, 256, 4]
IN_PG = [1, 10, 14, 18]
BPC, NCORES = 8, 8

N_OUT = [1030, 772, 516, 258]
PADW = [1036, 776, 520, 260]

# threshold knobs per stage 1..3
TH_ITERS = [13, 13, 12]
TH_ROUNDS = [2, 2, 2]
TH_OFF = [7, 7, 7]
SLACK = 24

# chunking tables: (stage): list of (Kin, Mout); plus input row ranges
CH1 = [(128, 120), (128, 120), (128, 80)]   # K is paired (64 base + 64 shifted)
CH2 = [(120, 84), (120, 84), (80, 56)]
CH3 = [(112, 72), (112, 72)]

_CACHE = {}


def _fuse_stage(w, b):
    O, I, K = w.shape
    opg_groups = O  # placeholder
    return None


def _fuse(w, b, groups):
    """fold(conv(x)) weights: W [taps][C_in, C_outF], bias bf [C_outF]."""
    O, I, K = w.shape
    C_in = groups * I
    opg = O // groups
    C_of = O // 2
    Wf = np.zeros((K, C_in, C_of), np.float32)
    bf = np.zeros(C_of, np.float32)
    for u in range(O):
        g, f = u // opg, u % opg
        cof = (g // 2) * opg + f
        for i in range(I):
            Wf[:, g * I + i, cof] += w[u, i, :]
        bf[cof] += b[u]
    return Wf, bf


def _host_prep(inputs):
    w = {}
    W1, b1 = _fuse(inputs["w1"], inputs["b1"], GROUPS[0])
    pairs = [(0, 1), (2, 3), (4, 5), (6, None)]
    for ci in range(3):
        c0, c1 = ci * 120, min(320, ci * 120 + 120)
        M = c1 - c0
        a = np.zeros((128, 4, 120), np.float32)
        for pi, (ta, tb) in enumerate(pairs):
            a[0:64, pi, :M] = W1[ta][:, c0:c1]
            if tb is not None:
                a[64:128, pi, :M] = W1[tb][:, c0:c1]
        w[f"w1_{ci}"] = a
        bb = np.zeros((1, 120), np.float32)
        bb[0, :M] = b1[c0:c1]
        w[f"b1_{ci}"] = bb

    W2, b2 = _fuse(inputs["w2"], inputs["b2"], GROUPS[1])
    for ci, (i0, o0, Ki, M) in enumerate(((0, 0, 120, 84), (120, 84, 120, 84), (240, 168, 80, 56))):
        a = np.zeros((120, 5, 84), np.float32)
        a[:Ki, :, :M] = W2[:, i0:i0 + Ki, o0:o0 + M].transpose(1, 0, 2)
        w[f"w2_{ci}"] = a
        bb = np.zeros((1, 84), np.float32)
        bb[0, :M] = b2[o0:o0 + M]
        w[f"b2_{ci}"] = bb

    W3, b3 = _fuse(inputs["w3"], inputs["b3"], GROUPS[2])
    for ci in range(2):
        a = W3[:, ci * 112:(ci + 1) * 112, ci * 72:(ci + 1) * 72].transpose(1, 0, 2).copy()
        w[f"w3_{ci}"] = np.ascontiguousarray(a)
        w[f"b3_{ci}"] = b3[ci * 72:(ci + 1) * 72].reshape(1, 72).copy()

    W4, b4 = _fuse(inputs["w4"], inputs["b4"], GROUPS[3])
    for si in range(2):
        w[f"w4_{si}"] = np.ascontiguousarray(W4[:, si * 72:(si + 1) * 72, :].transpose(1, 0, 2))
    w["b4_0"] = b4.reshape(1, 88).copy()

    fcw = inputs["fcw"]
    fcw4 = np.zeros((88, 4, 6), np.float32)
    for c in range(88):
        for j in range(4):
            fcw4[c, j] = fcw[:, 4 * c + j]
    w["fcw4"] = fcw4
    w["fcb"] = inputs["fcb"].reshape(NCLS, 1).astype(np.float32)
    w["emb"] = np.ascontiguousarray(inputs["emb"], np.float32)
    return w


_WSHAPES = {
    "w1_0": (128, 4, 120), "w1_1": (128, 4, 120), "w1_2": (128, 4, 120),
    "w2_0": (120, 5, 84), "w2_1": (120, 5, 84), "w2_2": (120, 5, 84),
    "w3_0": (112, 5, 72), "w3_1": (112, 5, 72),
    "w4_0": (72, 3, 88), "w4_1": (72, 3, 88),
    "b1_0": (1, 120), "b1_1": (1, 120), "b1_2": (1, 120),
    "b2_0": (1, 84), "b2_1": (1, 84), "b2_2": (1, 84),
    "b3_0": (1, 72), "b3_1": (1, 72), "b4_0": (1, 88),
    "fcw4": (88, 4, 6), "fcb": (NCLS, 1),
}


def _ceil16(x):
    return ((x + 15) // 16) * 16


def _build(debug=False):
    import concourse.bacc as bacc
    import concourse.mybir as mybir
    from concourse.bass import IndirectOffsetOnAxis
    from concourse.tile import TileContext
    from concourse.masks import make_identity
    from contextlib import ExitStack

    f32, u16, i16, i32 = (mybir.dt.float32, mybir.dt.uint16, mybir.dt.int16,
                          mybir.dt.int32)
    AF = mybir.ActivationFunctionType
    OP = mybir.AluOpType
    AX = mybir.AxisListType

    nc = bacc.Bacc("TRN2", target_bir_lowering=False, debug=False)

    tok = nc.dram_tensor("tok", [BPC, SEQ], i32, kind="ExternalInput")
    emb = nc.dram_tensor("emb", [V, E], f32, kind="ExternalInput")
    dten = {k: nc.dram_tensor(k, list(s), f32, kind="ExternalInput")
            for k, s in _WSHAPES.items()}
    out = nc.dram_tensor("out", [BPC, NCLS], f32, kind="ExternalOutput")
    dbg = {}
    if debug:
        for nm, sh in (("x1", [128, PADW[0]]), ("c1", [120, N_OUT[0]]),
                       ("x2", [120, PADW[1]]), ("c2", [84, N_OUT[1]]),
                       ("x3", [112, PADW[2]]), ("x4", [72, PADW[3]]),
                       ("x4f", [88, 4])):
            dbg[nm] = nc.dram_tensor("dbg_" + nm, sh, f32, kind="ExternalOutput")

    es = ExitStack()
    with TileContext(nc) as tc:
        wpool = es.enter_context(tc.tile_pool(name="w", bufs=1))
        xpool = es.enter_context(tc.tile_pool(name="x", bufs=2))
        kpool = es.enter_context(tc.tile_pool(name="k", bufs=2))
        spool = es.enter_context(tc.tile_pool(name="s", bufs=2))
        psc = es.enter_context(tc.tile_pool(name="psc", bufs=1, space="PSUM"))
        pse = es.enter_context(tc.tile_pool(name="pse", bufs=2, space="PSUM"))

        ident = wpool.tile([128, 128], f32, tag="ident")
        make_identity(nc, ident[:])
        it32 = wpool.tile([128, 16], i32, tag="iota16i")
        nc.gpsimd.iota(it32[:], pattern=[[1, 16]], base=0, channel_multiplier=0)
        iota16 = wpool.tile([128, 16], f32, tag="iota16")
        nc.vector.tensor_copy(iota16[:], it32[:])
        ones = wpool.tile([1, 1030], f32, tag="ones")
        nc.vector.memset(ones[:], 1.0)

        wt = {}
        for k, s in _WSHAPES.items():
            wt[k] = wpool.tile(list(s), f32, tag=k, name=k)
            nc.sync.dma_start(wt[k][:], dten[k][:])

        tokt = wpool.tile([128, BPC, 8], i32, tag="tok")
        nc.sync.dma_start(tokt[:], tok[:].rearrange("b (c p) -> p b c", p=128))

        fcps = pse.tile([NCLS, BPC], f32, tag="fcps")

        # ---------- compaction ----------
        def compact(xs, M, n, k, tf, dst, dst_off):
            M16 = _ceil16(M)
            mask = kpool.tile([128, 1032], f32, tag="mask")
            nc.vector.tensor_scalar(mask[:M, :n], xs[:M, :n], tf[:M, 0:1], None,
                                    op0=OP.is_ge)
            pp = kpool.tile([128, 1032], f32, tag="pp")
            nc.vector.tensor_tensor_scan(pp[:M, :n], mask[:M, :n], mask[:M, :n],
                                         0.0, op0=OP.add, op1=OP.bypass)
            idx = kpool.tile([128, 1032], i16, tag="idx")
            if M16 > M:
                st = max(q for q in (0, 32, 64, 96) if q <= M)
                nc.vector.memset(idx[st:M16, :n], 0)
            nc.vector.tensor_tensor(idx[:M, :n], mask[:M, :n], pp[:M, :n],
                                    op=OP.mult)
            xu = xs[:].bitcast(u16)
            phi = kpool.tile([128, 1032], u16, tag="phi")
            plo = kpool.tile([128, 1032], u16, tag="plo")
            nc.gpsimd.tensor_copy(phi[:M16, :n], xu[:M16, 1:2 * n:2])
            nc.gpsimd.tensor_copy(plo[:M16, :n], xu[:M16, 0:2 * n:2])
            nslots = k + SLACK
            lhi = kpool.tile([128, KPOOL[0] + SLACK], u16, tag="lhi")
            llo = kpool.tile([128, KPOOL[0] + SLACK], u16, tag="llo")
            nc.gpsimd.local_scatter(lhi[:M16, :nslots], phi[:M16, :n],
                                    idx[:M16, :n], channels=M16,
                                    num_elems=nslots, num_idxs=n)
            nc.gpsimd.local_scatter(llo[:M16, :nslots], plo[:M16, :n],
                                    idx[:M16, :n], channels=M16,
                                    num_elems=nslots, num_idxs=n)
            du = dst[:, dst_off:dst_off + k].bitcast(u16)
            nc.scalar.copy(du[:M, 1:2 * k:2], lhi[:M, 1:k + 1])
            nc.scalar.copy(du[:M, 0:2 * k:2], llo[:M, 1:k + 1])
            nc.scalar.activation(dst[:M, dst_off:dst_off + k],
                                 dst[:M, dst_off:dst_off + k], AF.Tanh)

        # ---------- grouped threshold search over a stage's chunks ----------
        def kmax_group(xss, Ms, n, k, st, dsts, dst_off):
            NCH = len(xss)
            ITERS, R, OFF = TH_ITERS[st], TH_ROUNDS[st], TH_OFF[st]
            TSIG = float(2 * (k - OFF) - n)
            S = n // 8
            tL = spool.tile([128, 4], f32, tag="tL")
            tH = spool.tile([128, 4], f32, tag="tH")
            ssum = spool.tile([128, 4], f32, tag="ssum")
            nbias = spool.tile([128, 4], f32, tag="nbias")
            hi = spool.tile([128, 4], mybir.dt.uint8, tag="hi")
            lo = spool.tile([128, 4], mybir.dt.uint8, tag="lo")
            acc = spool.tile([128, 4], f32, tag="acc")
            smp = kpool.tile([128, 136], f32, tag="smp")
            nsmp = kpool.tile([128, 136], f32, tag="nsmp")
            m8 = kpool.tile([128, 8], f32, tag="m8")
            sg = kpool.tile([128, 1032], f32, tag="sg")
            for c, (xs, M) in enumerate(zip(xss, Ms)):
                nc.vector.tensor_copy(smp[:M, :S], xs[:M, 0:8 * S:8])
                nc.vector.tensor_scalar_mul(nsmp[:M, :S], smp[:M, :S], -1.0)
                nc.vector.max(out=m8[:M], in_=nsmp[:M, :S])
                nc.vector.tensor_scalar_mul(tL[:M, c:c + 1], m8[:M, 7:8], -1.0)
                nc.vector.max(out=m8[:M], in_=smp[:M, :S])
                nc.vector.tensor_copy(tH[:M, c:c + 1], m8[:M, 0:1])
            for it in range(ITERS):
                # pivot (tL+tH)/2 via scale=2 trick: sign(2x - (tL+tH))
                nc.vector.tensor_tensor(ssum[:, :NCH], tL[:, :NCH], tH[:, :NCH],
                                        op=OP.add)
                nc.vector.tensor_scalar_mul(nbias[:, :NCH], ssum[:, :NCH], -1.0)
                for c, (xs, M) in enumerate(zip(xss, Ms)):
                    nc.scalar.activation(sg[:M, :n], xs[:M, :n], AF.Sign,
                                         bias=nbias[:M, c:c + 1], scale=2.0,
                                         accum_out=acc[:M, c:c + 1])
                nc.vector.tensor_scalar(hi[:, :NCH], acc[:, :NCH], TSIG, None,
                                        op0=OP.is_le)
                nc.vector.tensor_scalar(lo[:, :NCH], acc[:, :NCH], TSIG, None,
                                        op0=OP.is_gt)
                # mid = 0.5*ssum; tH = mid where hi; tL = mid where !hi
                nc.vector.tensor_scalar_mul(ssum[:, :NCH], ssum[:, :NCH], 0.5)
                nc.vector.copy_predicated(tH[:, :NCH], hi[:, :NCH],
                                          ssum[:, :NCH])
                nc.vector.copy_predicated(tL[:, :NCH], lo[:, :NCH],
                                          ssum[:, :NCH])
            # final count at tH
            nc.vector.tensor_scalar_mul(nbias[:, :NCH], tH[:, :NCH], -1.0)
            for c, (xs, M) in enumerate(zip(xss, Ms)):
                nc.scalar.activation(sg[:M, :n], xs[:M, :n], AF.Sign,
                                     bias=nbias[:M, c:c + 1], scale=1.0,
                                     accum_out=acc[:M, c:c + 1])
            jj = spool.tile([128, 4], f32, tag="jj")
            nc.vector.tensor_scalar(jj[:, :NCH], acc[:, :NCH], -0.5,
                                    float(k) - n / 2.0, op0=OP.mult, op1=OP.add)
            nc.vector.tensor_scalar_max(jj[:, :NCH], jj[:, :NCH], 0.0)
            nc.vector.tensor_scalar_min(jj[:, :NCH], jj[:, :NCH],
                                        float(8 * R - 1))
            # snap per chunk
            for c, (xs, M) in enumerate(zip(xss, Ms)):
                mlt = kpool.tile([128, 1032], f32, tag="mask")
                nc.vector.tensor_scalar(mlt[:M, :n], xs[:M, :n], tH[:M, c:c + 1],
                                        None, op0=OP.is_ge)
                wv = kpool.tile([128, 1032], f32, tag="wv")
                nc.vector.scalar_tensor_tensor(wv[:M, :n], mlt[:M, :n], -1e30,
                                               xs[:M, :n], op0=OP.mult,
                                               op1=OP.add)
                cand = kpool.tile([128, 16], f32, tag="cand")
                nc.vector.tensor_copy(cand[:M, 0:1], tH[:M, c:c + 1])
                w8 = kpool.tile([128, 8], f32, tag="w8")
                for r in range(R):
                    nc.vector.max(out=w8[:M], in_=wv[:M, :n])
                    ncols = 8 if r < R - 1 else 7
                    nc.vector.tensor_copy(cand[:M, 8 * r + 1:8 * r + 1 + ncols],
                                          w8[:M, 0:ncols])
                    if r < R - 1:
                        nc.vector.match_replace(out=wv[:M, :n],
                                                in_to_replace=w8[:M],
                                                in_values=wv[:M, :n],
                                                imm_value=-1e30)
                oh = kpool.tile([128, 16], f32, tag="oh")
                nc.vector.tensor_scalar(oh[:M, :8 * R], iota16[:M, :8 * R],
                                        jj[:M, c:c + 1], None, op0=OP.is_equal)
                nc.vector.tensor_tensor(oh[:M, :8 * R], oh[:M, :8 * R],
                                        cand[:M, :8 * R], op=OP.mult)
                tf = spool.tile([128, 1], f32, tag="tf")
                nc.vector.tensor_reduce(tf[:M], oh[:M, :8 * R], axis=AX.X,
                                        op=OP.add)
                compact(xs, M, n, k, tf, dsts[c], dst_off)

        # ---------- per-batch pipeline ----------
        for b in range(BPC):
            X1 = xpool.tile([128, PADW[0]], f32, tag="X1")
            nc.vector.memset(X1[:, 0:6], 0.0)
            nc.vector.memset(X1[:, 6 + SEQ:], 0.0)
            nc.vector.memset(X1[64:128, 5 + SEQ:6 + SEQ], 0.0)
            gat = xpool.tile([128, 8 * E], f32, tag="gat")
            for c in range(8):
                nc.gpsimd.indirect_dma_start(
                    out=gat[:, c * E:(c + 1) * E], out_offset=None, in_=emb[:, :],
                    in_offset=IndirectOffsetOnAxis(ap=tokt[:, b, c:c + 1], axis=0))
            for h in range(2):
                ps = pse.tile([E, 512], f32, tag="pse")
                for q in range(4):
                    c = h * 4 + q
                    nc.tensor.transpose(out=ps[:, q * 128:(q + 1) * 128],
                                        in_=gat[:, c * E:(c + 1) * E],
                                        identity=ident[:])
                nc.scalar.copy(X1[0:64, 6 + 512 * h:6 + 512 * (h + 1)], ps[:, :])
                nc.scalar.copy(X1[64:128, 5 + 512 * h:5 + 512 * (h + 1)], ps[:, :])
            if debug and b == 0:
                nc.sync.dma_start(dbg["x1"][:], X1[:])

            # ---- stage 1 ----
            xss, dsts, Ms = [], [], []
            for ci, (Kp_, Mo) in enumerate(CH1):
                ps = psc.tile([128, 1536], f32, tag="psc")
                nchunks1 = ((0, 512), (512, 1024), (1024, 1030))
                for (n0, n1) in nchunks1:
                    nc.tensor.matmul(ps[:120, n0:n1], wt[f"b1_{ci}"][:, :],
                                     ones[0:1, n0:n1], start=True, stop=False)
                for pi in range(4):
                    Kp = 128 if pi < 3 else 64
                    lhs = wt[f"w1_{ci}"][0:Kp, pi, :]
                    for (n0, n1) in nchunks1:
                        nc.tensor.matmul(ps[:120, n0:n1], lhs,
                                         X1[0:Kp, 2 * pi + n0:2 * pi + n1],
                                         start=False, stop=(pi == 3))
                xs = kpool.tile([128, 1032], f32, tag=f"xs{ci}")
                nc.scalar.copy(xs[:Mo, :1030], ps[:Mo, 0:1030])
                if debug and b == 0 and ci == 0:
                    nc.sync.dma_start(dbg["c1"][:], xs[:120, :1030])
                xt = xpool.tile([128, PADW[1]], f32, tag=f"X2_{ci}")
                nc.vector.memset(xt[:Mo, 0:4], 0.0)
                nc.vector.memset(xt[:Mo, 4 + KPOOL[0]:], 0.0)
                xss.append(xs); dsts.append(xt); Ms.append(Mo)
            kmax_group(xss, Ms, 1030, KPOOL[0], 0, dsts, 4)
            X2 = dsts
            if debug and b == 0:
                nc.sync.dma_start(dbg["x2"][:], X2[0][:120, :])

            # ---- stage 2 ----
            xss, dsts, Ms = [], [], []
            for ci, (Ki, Mo) in enumerate(CH2):
                ps = psc.tile([128, 1536], f32, tag="psc")
                nchunks2 = ((0, 512), (512, 772))
                for (n0, n1) in nchunks2:
                    nc.tensor.matmul(ps[:84, n0:n1], wt[f"b2_{ci}"][:, :],
                                     ones[0:1, n0:n1], start=True, stop=False)
                for t in range(5):
                    lhs = wt[f"w2_{ci}"][0:Ki, t, :]
                    for (n0, n1) in nchunks2:
                        nc.tensor.matmul(ps[:84, n0:n1], lhs,
                                         X2[ci][0:Ki, t + n0:t + n1],
                                         start=False, stop=(t == 4))
                xs = kpool.tile([128, 1032], f32, tag=f"xs{ci}")
                nc.scalar.copy(xs[:Mo, :772], ps[:Mo, 0:772])
                if debug and b == 0 and ci == 0:
                    nc.sync.dma_start(dbg["c2"][:], xs[:84, :772])
                yt = xpool.tile([128, PADW[2]], f32, tag=f"Y3_{ci}")
                nc.vector.memset(yt[:Mo, 0:4], 0.0)
                nc.vector.memset(yt[:Mo, 4 + KPOOL[1]:], 0.0)
                xss.append(xs); dsts.append(yt); Ms.append(Mo)
            kmax_group(xss, Ms, 772, KPOOL[1], 1, dsts, 4)
            Y3 = dsts

            X3 = [xpool.tile([128, PADW[2]], f32, tag=f"X3_{i}", name=f"X3_{i}") for i in range(2)]
            nc.sync.dma_start(X3[0][0:84, :], Y3[0][0:84, :])
            nc.scalar.dma_start(X3[0][84:112, :], Y3[1][0:28, :])
            nc.scalar.dma_start(X3[1][0:56, :], Y3[1][28:84, :])
            nc.gpsimd.dma_start(X3[1][56:112, :], Y3[2][0:56, :])
            if debug and b == 0:
                nc.sync.dma_start(dbg["x3"][:], X3[0][:112, :])

            # ---- stage 3 ----
            xss, dsts, Ms = [], [], []
            for ci in range(2):
                ps = psc.tile([128, 1536], f32, tag="psc")
                nchunks3 = ((0, 512), (512, 516))
                for (n0, n1) in nchunks3:
                    nc.tensor.matmul(ps[:72, n0:n1], wt[f"b3_{ci}"][:, :],
                                     ones[0:1, n0:n1], start=True, stop=False)
                for t in range(5):
                    lhs = wt[f"w3_{ci}"][0:112, t, :]
                    for (n0, n1) in nchunks3:
                        nc.tensor.matmul(ps[:72, n0:n1], lhs,
                                         X3[ci][0:112, t + n0:t + n1],
                                         start=False, stop=(t == 4))
                xs = kpool.tile([128, 1032], f32, tag=f"xs{ci}")
                nc.scalar.copy(xs[:72, :516], ps[:72, 0:516])
                xt = xpool.tile([128, PADW[3]], f32, tag=f"X4_{ci}")
                nc.vector.memset(xt[:72, 0:2], 0.0)
                nc.vector.memset(xt[:72, 2 + KPOOL[2]:], 0.0)
                xss.append(xs); dsts.append(xt); Ms.append(72)
            kmax_group(xss, Ms, 516, KPOOL[2], 2, dsts, 2)
            X4 = dsts
            if debug and b == 0:
                nc.sync.dma_start(dbg["x4"][:], X4[0][:72, :])

            # ---- stage 4 + fc ----
            ps = psc.tile([128, 1536], f32, tag="psc")
            nc.tensor.matmul(ps[:88, 0:258], wt["b4_0"][:, :], ones[0:1, 0:258],
                             start=True, stop=False)
            for t in range(3):
                for si in range(2):
                    nc.tensor.matmul(ps[:88, 0:258],
                                     wt[f"w4_{si}"][0:72, t, :],
                                     X4[si][0:72, t:t + 258],
                                     start=False, stop=(t == 2 and si == 1))
            xs = kpool.tile([128, 1032], f32, tag="xs0")
            nc.scalar.copy(xs[:88, :258], ps[:88, 0:258])
            m8 = kpool.tile([128, 8], f32, tag="m8")
            nc.vector.max(out=m8[:88], in_=xs[:88, :258])
            tf4 = spool.tile([128, 1], f32, tag="tf")
            nc.vector.tensor_copy(tf4[:88], m8[:88, 3:4])
            x4f = xpool.tile([128, 8], f32, tag="x4f")
            compact(xs, 88, 258, 4, tf4, x4f, 0)
            if debug and b == 0:
                nc.sync.dma_start(dbg["x4f"][:], x4f[:88, 0:4])

            for j in range(4):
                nc.tensor.matmul(fcps[:, b:b + 1], wt["fcw4"][:, j, :],
                                 x4f[:88, j:j + 1], start=(j == 0),
                                 stop=(j == 3))

        fcs = wpool.tile([NCLS, BPC], f32, tag="fcs")
        nc.vector.tensor_scalar(fcs[:], fcps[:], wt["fcb"][:, 0:1], None,
                                op0=OP.add)
        nc.sync.dma_start(out[:].rearrange("b o -> o b"), fcs[:])
        es.close()
    nc.compile()
    return nc


def kernel(**inputs):
    if "nc" not in _CACHE:
        _CACHE["nc"] = _build(debug=False)
    nc = _CACHE["nc"]
    w = _host_prep(inputs)
    tokens = np.ascontiguousarray(np.asarray(inputs["tokens"]).astype(np.int32))
    in_maps = []
    for c in range(NCORES):
        m = {k: w[k] for k in w}
        m["tok"] = tokens[c * BPC:(c + 1) * BPC]
        in_maps.append(m)
    from concourse import bass_utils
    res = bass_utils.run_bass_kernel_spmd(nc, in_maps, core_ids=list(range(NCORES)))
    return np.concatenate([r["out"] for r in res.results], axis=0).astype(np.float32)
